# revision 38
# baseline (speedup 1.0000x reference)
"""Bidirectional Mamba (MixerModel) Trainium2 kernel.

Sharding: data-parallel over batch. 8 batch elements -> 8 NeuronCores.
Each core runs the full 2-direction x 4-layer model for its batch element
(no collectives; the backward direction consumes a host-flipped copy of the
input, and the softmax attention pool is order-invariant so the backward
output never needs unflipping). Host stacks the per-core [64] outputs.

On-chip layout is feature-major: activations live as [feature, T] tiles so
the selective-scan recurrence h_t = dA_t * h_{t-1} + dBx_t maps onto the
DVE tensor_tensor_scan instruction (d_inner=128 on partitions, one scan
per state s=0..15). T=2048 is one chunk (no carries, no conv tail).

Decay factors via the model's exact A_s = -(s+1): da_s = exp(A_s*dt)
= w^(s+1) with w = sigmoid(-(dt_raw+dt_b)), so the 16 per-state decays
are built by repeated Act-engine squaring along a DFS of the power tree
(4 retention slots) plus 7 odd-power muls. Square/Copy live in EVERY
activation-table set, so the scan never forces a table reload
(ACT_TABLE_LOAD is 1.3us); the only per-layer excursion is one Ln for
u = dt*x = (-ln w)*x.

Schedule: the two direction streams are software-pipelined against each
other. Engine queues are in-order, so a lone serial stage chain starves
the machine; the emission order is

    ... scan(d0,l) [zipped with pre(d1,l)] ; post(d0,l) ;
        scan(d1,l) [zipped with post(d0,l), pre(d0,l+1)] ; ...

i.e. while one direction's 16-state scan occupies DVE/PE/Pool/Act, the
other direction's LN/in_proj/conv/x_proj closures are drained one per
state into the same queues. Within a scan, states are pipelined one
ahead (B-broadcast and scan of the next state issue before the C-side
of the current) so the PE never stalls behind the DVE.

Engine assignment (cost-model rates: DVE f32 1.04 / bf16 0.52 ns/col,
Act 0.93, Pool 2.03, PE bf16 0.83 ns/col): projections and one-hot row
broadcasts are bf16 PE matmuls into PSUM; the u*B / h*C muls either
read PSUM at DVE f32 rate ("psum" states) or are first evacuated
PSUM->SBUF bf16 by the Act engine — the real HW forbids GPSIMD PSUM
reads — and then multiplied on Pool or DVE-bf16 (MUL_MODE per state).
y = sum_s C_s*h_s accumulates on the PE via identity-matmul into a
[128,2048] PSUM tile. PSUM = py (4 banks) + 4x [128,512] "bc" ring
(4 banks) = exactly 8 banks.
"""

import numpy as np

D_MODEL = 64
N_LAYER = 4
D_INNER = 128
D_STATE = 16
D_CONV = 4
DT_RANK = 4
EPS = 1e-5
T = 2048
B = 8
NCORES = 8
MM = 512               # max matmul free dim (one PSUM bank)

# per-state mul routing for the scan's u*B / h*C products. The real HW
# forbids GPSIMD reads from PSUM, so Pool-assigned states first evacuate
# the PE broadcast PSUM->SBUF bf16 on the Act engine ("evac"), which also
# lets DVE run those muls at its 2x bf16 rate.
#   "psum": DVE mul reads the broadcast from PSUM (f32 rate, no evac)
#   "pool": Act evac -> Pool mul (SBUF)
#   "dve16": Act evac -> DVE mul (bf16 rate)
import os as _os
_cfg = _os.environ.get("BK_MULMODE", "psdpsdpsdpsdpsdp")
# per-state char: p=psum(DVE f32), s=pool(evac+Pool), d=dve16(evac+DVE bf16)
MUL_MODE = {_s: {"p": "psum", "s": "pool", "d": "dve16"}[_cfg[_s]]
            for _s in range(D_STATE)}


def _legalize_sync_waits(nc, mybir, maxw=1):
    """This container's walrus only accepts one sync-wait command per
    instruction (newer bass emits several, e.g. on the kernel-tail drain).
    Split excess waits onto preceding same-engine NOPs — semantically
    identical: the engine blocks on each wait in turn before the original
    instruction issues."""
    for blk in nc.m.functions[0].blocks:
        newlist, changed = [], False
        for inst in blk.instructions:
            si = inst.sync_info
            waits = list(si.on_wait) if si and si.on_wait else []
            if len(waits) > maxw:
                k = 0
                while len(waits) > maxw:
                    chunk, waits = waits[:maxw], waits[maxw:]
                    newlist.append(mybir.InstNoOp(
                        name=f"{inst.name}-waitsplit{k}", engine=inst.engine,
                        sync_info=mybir.SyncInfo(on_wait=chunk, on_update=[])))
                    k += 1
                inst.sync_info = mybir.SyncInfo(
                    on_wait=waits, on_update=list(si.on_update or []))
                changed = True
            newlist.append(inst)
        if changed:
            blk.instructions = newlist


def build_nc(legalize=True, reps=1):
    import concourse.bass as bass
    import concourse.bacc as bacc
    import concourse.mybir as mybir
    import concourse.tile as tile
    from contextlib import ExitStack

    dt32 = mybir.dt.float32
    dt16 = mybir.dt.bfloat16
    Alu = mybir.AluOpType
    Act = mybir.ActivationFunctionType

    # NOTE: bacc.Bacc's finalize pipeline emits event-semaphore/register
    # constructs this container's walrus rejects ("Reg has not been
    # allocated"), so we stay on plain Bass + _legalize_sync_waits.
    nc = bass.Bass("TRN2", target_bir_lowering=False, debug=False,
                   num_devices=NCORES)

    def din(name, shape, dt=dt32):
        return nc.dram_tensor(name, list(shape), dt, kind="ExternalInput").ap()

    xin = din("xin", (2, D_MODEL, T))           # fwd + flipped input
    in_wT = din("in_wT", (2, N_LAYER, D_MODEL, 2 * D_INNER), dt16)
    conv_w = din("conv_w", (2, N_LAYER, D_INNER, D_CONV))
    conv_b = din("conv_b", (2, N_LAYER, D_INNER, 1))
    xproj_wTp = din("xproj_wTp", (2, N_LAYER, D_INNER, 68), dt16)
    dt_wT = din("dt_wT", (2, N_LAYER, DT_RANK, D_INNER), dt16)
    dt_b = din("dt_b", (2, N_LAYER, D_INNER, 1))
    D_in = din("Dp", (2, N_LAYER, D_INNER, 1))
    out_wT = din("out_wT", (2, N_LAYER, D_INNER, D_MODEL), dt16)
    nw_in = din("nw", (2, N_LAYER, D_MODEL, 1))
    nb_in = din("nb", (2, N_LAYER, D_MODEL, 1))
    nf_w = din("nf_w", (D_MODEL, 1))
    nf_b = din("nf_b", (D_MODEL, 1))
    pool_wT = din("pool_wT", (2, D_MODEL, 1), dt16)
    ll_wT2 = din("ll_wT2", (2, D_MODEL, D_MODEL), dt16)
    ll_b = din("ll_b", (D_MODEL, 1))
    ident_in = din("ident", (D_INNER, D_INNER), dt16)
    selmat_in = din("selmat", (48, D_STATE * D_INNER), dt16)

    out_d = nc.dram_tensor("out", [D_MODEL, 1], dt32, kind="ExternalOutput").ap()

    with tile.TileContext(nc) as tc, ExitStack() as ctx:
        const = ctx.enter_context(tc.tile_pool(name="const", bufs=1))
        sb = ctx.enter_context(tc.tile_pool(name="sb", bufs=2))
        s1 = ctx.enter_context(tc.tile_pool(name="s1", bufs=1))
        scn = ctx.enter_context(tc.tile_pool(name="scn", bufs=2))
        scn2 = ctx.enter_context(tc.tile_pool(name="scn2", bufs=2))
        rows = ctx.enter_context(tc.tile_pool(name="rows", bufs=1))
        hrow = ctx.enter_context(tc.tile_pool(name="hrow", bufs=1))
        # PSUM: 4-deep ring of [128,512] tiles (1 bank each) for every
        # projection output / broadcast, + the [128,2048] y-accumulator.
        ps = ctx.enter_context(tc.tile_pool(name="ps", bufs=4, space="PSUM"))
        py = ctx.enter_context(tc.tile_pool(name="py", bufs=1, space="PSUM"))

        def bc(name):
            return ps.tile([D_INNER, MM], dt32, tag="bc", name=name)

        ones_row = const.tile([1, D_MODEL], dt16, tag="ones_row")
        nc.vector.memset(ones_row, 1.0)
        lnsel = const.tile([D_MODEL, 1], dt16, tag="lnsel")
        nc.vector.memset(lnsel, 1.0 / D_MODEL)
        eps_c = const.tile([1, 1], dt32, tag="epsc")
        nc.vector.memset(eps_c, EPS)
        negone = const.tile([D_INNER, 1], dt32, tag="negone")
        nc.vector.memset(negone, -1.0)

        _dmaq = [0]

        def cload(tag, ap_src, shape, dt=dt32):
            t = const.tile(list(shape), dt, tag=tag)
            # alternate the two HWDGE trigger queues (SP / Activation)
            q = nc.sync if _dmaq[0] % 2 == 0 else nc.scalar
            _dmaq[0] += 1
            q.dma_start(out=t, in_=ap_src)
            return t

        # rep-0 input load first: the first LN depends on it, and the bulk
        # weight DMAs would otherwise delay kernel start by ~50us.
        resb0 = sb.tile([2 * D_MODEL, T], dt32, tag="res", name="resb0")
        for d in range(2):
            nc.sync.dma_start(out=resb0[d * D_MODEL:(d + 1) * D_MODEL, :],
                              in_=xin[d])

        P = {}
        SI = {}
        for l in range(N_LAYER):
            if l == 1:
                SI["ident"] = const.tile([D_INNER, D_INNER], dt16, tag="ident", name="ident")
                nc.sync.dma_start(out=SI["ident"], in_=ident_in)
                SI["selmat"] = const.tile([48, D_STATE * D_INNER], dt16,
                                          tag="selmat", name="selmat")
                nc.scalar.dma_start(out=SI["selmat"], in_=selmat_in)
            for d in range(2):
                k = (d, l)
                P[("in_wT",) + k] = cload(f"in_wT{d}{l}", in_wT[d, l], (D_MODEL, 2 * D_INNER), dt16)
                P[("conv_w",) + k] = cload(f"conv_w{d}{l}", conv_w[d, l], (D_INNER, D_CONV))
                P[("conv_b",) + k] = cload(f"conv_b{d}{l}", conv_b[d, l], (D_INNER, 1))
                P[("xproj_wTp",) + k] = cload(f"xproj{d}{l}", xproj_wTp[d, l], (D_INNER, 68), dt16)
                P[("dt_wT",) + k] = cload(f"dtw{d}{l}", dt_wT[d, l], (DT_RANK, D_INNER), dt16)
                P[("dt_b",) + k] = cload(f"dt_b{d}{l}", dt_b[d, l], (D_INNER, 1))
                P[("Dp",) + k] = cload(f"Dp{d}{l}", D_in[d, l], (D_INNER, 1))
                P[("out_wT",) + k] = cload(f"out_wT{d}{l}", out_wT[d, l], (D_INNER, D_MODEL), dt16)
                P[("nw",) + k] = cload(f"nw{d}{l}", nw_in[d, l], (D_MODEL, 1))
                P[("nb",) + k] = cload(f"nb{d}{l}", nb_in[d, l], (D_MODEL, 1))
        ident = SI["ident"]
        selmat = SI["selmat"]
        nfw_sb = cload("nfw", nf_w, (D_MODEL, 1))
        nfb_sb = cload("nfb", nf_b, (D_MODEL, 1))
        pw_sb = [cload(f"pw{d}", pool_wT[d], (D_MODEL, 1), dt16) for d in range(2)]
        llw_sb = [cload(f"llw{d}", ll_wT2[d], (D_MODEL, D_MODEL), dt16) for d in range(2)]
        llb_sb = cload("llb", ll_b, (D_MODEL, 1))

        # ---- layernorm over features -> list of emission closures ------
        # src [64,T] f32 at base 0/64; writes hln bf16 (base 0).
        def ln_stages(src, nw_c, nb_c, hln_c, tp):
            def stat():
                src16 = s1.tile([D_MODEL, T], dt16, tag=f"lnsrc{tp}", name="src16")
                nc.scalar.activation(src16, src, Act.Copy)
                sq16 = s1.tile([D_MODEL, T], dt16, tag=f"lnsq{tp}", name="sq16")
                nc.scalar.activation(sq16, src, Act.Square)
                mean_sb = rows.tile([1, T], dt16, tag=f"mean{tp}")
                rstd_sb = rows.tile([1, T], dt16, tag=f"rstd{tp}")
                scr = rows.tile([1, T], dt32, tag=f"scr{tp}")
                for j in range(T // MM):
                    sj = slice(j * MM, (j + 1) * MM)
                    pm = bc("pmln")
                    nc.tensor.matmul(pm[0:1], lnsel, src16[:, sj],
                                     start=True, stop=True)
                    nc.tensor.matmul(pm[32:33], lnsel, sq16[:, sj],
                                     start=True, stop=True)
                    nc.scalar.activation(mean_sb[:, sj], pm[0:1], Act.Copy)
                    nc.scalar.activation(scr[:, sj], pm[32:33], Act.Copy)
                # rstd doubles as mean^2 scratch before its final value
                with nc.allow_low_precision("rstd bf16 feeds bf16 matmul"):
                    nc.scalar.activation(rstd_sb, mean_sb, Act.Square)
                    nc.vector.tensor_sub(scr, scr, rstd_sb)
                    nc.scalar.activation(scr, scr, Act.Sqrt, bias=eps_c)
                    nc.vector.reciprocal(rstd_sb, scr)
                return mean_sb, rstd_sb

            def norm(mr, j):
                mean_sb, rstd_sb = mr
                sj = slice(j * MM, (j + 1) * MM)
                mb = bc("mbln")
                nc.tensor.matmul(mb[0:D_MODEL], ones_row, mean_sb[:, sj],
                                 start=True, stop=True)
                rb = bc("rbln")
                nc.tensor.matmul(rb[0:D_MODEL], ones_row, rstd_sb[:, sj],
                                 start=True, stop=True)
                tmp = s1.tile([D_MODEL, MM], dt32, tag=f"lntmp{tp}")
                nc.vector.tensor_sub(tmp, src[:, sj], mb[0:D_MODEL])
                nc.vector.scalar_tensor_tensor(tmp, tmp, nw_c, rb[0:D_MODEL],
                                               op0=Alu.mult, op1=Alu.mult)
                nc.scalar.activation(hln_c[:, sj], tmp, Act.Identity, bias=nb_c)

            st = {}
            stages = [lambda: st.__setitem__('mr', stat())]
            for j in range(T // MM):
                stages.append(lambda j=j: norm(st['mr'], j))
            return stages

        # ---- per-direction layer state ---------------------------------
        LS = [{}, {}]

        # pre(d,l): LN -> in_proj -> conv -> x_proj -> dt -> u, as closures
        def pre_stages(d, l, res_old):
            S = LS[d]
            base = d * D_MODEL
            src = res_old[base:base + D_MODEL, :]
            stages = []

            hln = s1.tile([D_MODEL, T], dt16, tag=f"hln{d}", name=f"hln{d}")
            S["src"] = src
            stages += ln_stages(src, P[("nw", d, l)], P[("nb", d, l)], hln, f"p{d}")

            xpad = s1.tile([D_INNER, D_CONV - 1 + T], dt16, tag=f"xpad{d}",
                           name=f"xpad{d}")
            zsilu = s1.tile([D_INNER, T], dt16, tag=f"zsilu{d}", name=f"zsilu{d}")
            S["zsilu"] = zsilu
            wx = P[("in_wT", d, l)]

            def inproj(j):
                sj = slice(j * MM, (j + 1) * MM)
                px = bc("px")
                nc.tensor.matmul(px, wx[:, 0:D_INNER], hln[:, sj],
                                 start=True, stop=True)
                nc.scalar.activation(
                    xpad[:, D_CONV - 1 + j * MM:D_CONV - 1 + (j + 1) * MM],
                    px, Act.Copy)
                pz = bc("pz")
                nc.tensor.matmul(pz, wx[:, D_INNER:], hln[:, sj],
                                 start=True, stop=True)
                zsig = sb.tile([D_INNER, MM], dt16, tag="zsig")
                nc.scalar.activation(zsig, pz, Act.Sigmoid)
                nc.vector.tensor_mul(zsilu[:, sj], zsig, pz)

            for j in range(T // MM):
                stages.append(lambda j=j: inproj(j))

            cw = P[("conv_w", d, l)]
            cacc = s1.tile([D_INNER, T], dt16, tag=f"yo{d}", name=f"cacc{d}")
            xact = s1.tile([D_INNER, T], dt16, tag=f"xact{d}", name=f"xact{d}")
            S["xact"] = xact

            def conv_a():
                nc.vector.memset(xpad[:, 0:D_CONV - 1], 0.0)
                nc.gpsimd.tensor_scalar(cacc, xpad[:, 0:T], cw[:, 0:1],
                                        P[("conv_b", d, l)],
                                        op0=Alu.mult, op1=Alu.add)

            def conv_tap(jj):
                nc.vector.scalar_tensor_tensor(cacc, xpad[:, jj:jj + T],
                                               cw[:, jj:jj + 1], cacc,
                                               op0=Alu.mult, op1=Alu.add)

            def conv_c():
                xsig = s1.tile([D_INNER, T], dt16, tag=f"xsig{d}", name=f"xsig{d}")
                nc.scalar.activation(xsig, cacc, Act.Sigmoid)
                nc.gpsimd.tensor_mul(xact, cacc, xsig)

            stages.append(conv_a)
            for jj in range(1, D_CONV):
                stages.append(lambda jj=jj: conv_tap(jj))
            stages.append(conv_c)

            bcs = s1.tile([48, T], dt16, tag=f"bcs{d}", name=f"bcs{d}")
            dtr = s1.tile([DT_RANK, T], dt16, tag=f"dtr{d}", name=f"dtr{d}")
            S["bcs"] = bcs

            def xproj(j):
                sj = slice(j * MM, (j + 1) * MM)
                pd_ = bc("pd")
                nc.tensor.matmul(pd_[0:68], P[("xproj_wTp", d, l)], xact[:, sj],
                                 start=True, stop=True)
                nc.scalar.activation(bcs[:, sj], pd_[0:48], Act.Copy)
                nc.scalar.activation(dtr[:, sj], pd_[64:68], Act.Copy)

            for j in range(T // MM):
                stages.append(lambda j=j: xproj(j))

            # w = sigmoid(-(dt_raw + dt_b)) = exp(-softplus(dt_raw + dt_b))
            # = exp(-dt): the per-state decay is da_s = w^(s+1) (A_s = -(s+1)
            # exactly in this model), built by repeated squaring -- Square is
            # in EVERY act-table set, so the scan causes no table reloads.
            # dt_b arrives negated from the host.
            w = s1.tile([D_INNER, T], dt16, tag=f"w{d}", name=f"w{d}")
            S["w"] = w

            def dtproj(j):
                sj = slice(j * MM, (j + 1) * MM)
                pt = bc("pt")
                nc.tensor.matmul(pt, P[("dt_wT", d, l)], dtr[:, sj],
                                 start=True, stop=True)
                nc.scalar.activation(w[:, sj], pt, Act.Sigmoid, scale=-1.0,
                                     bias=P[("dt_b", d, l)])

            for j in range(T // MM):
                stages.append(lambda j=j: dtproj(j))

            def umul():
                # u = dt * x = (-ln w) * x  (the layer's one Ln excursion)
                u = s1.tile([D_INNER, T], dt16, tag=f"u{d}", name=f"u{d}")
                nc.scalar.activation(u, w, Act.Ln)
                nc.vector.scalar_tensor_tensor(u, u, negone, xact,
                                               op0=Alu.mult, op1=Alu.mult)
                S["u"] = u

            stages.append(umul)
            return stages

        # scan(d,l): 16-state selective scan, one state pipelined ahead.
        # `extra` closures (the other stream's stages) are drained one per
        # state so both streams share the engine queues.
        # da_s = w^(s+1): power tree visited in DFS order so each tile is
        # derived from a live parent by Act.Square (universal table set) or
        # a DVE/Pool mul with w. 4 retention slots cover peak liveness.
        PORDER = [1, 2, 4, 8, 16, 9, 5, 10, 11, 3, 6, 12, 13, 7, 14, 15]
        PDERIV = {2: (1, 'sq'), 4: (2, 'sq'), 8: (4, 'sq'), 16: (8, 'sq'),
                  9: (8, 'mul'), 5: (4, 'mul'), 10: (5, 'sq'), 11: (10, 'mul'),
                  3: (2, 'mul'), 6: (3, 'sq'), 12: (6, 'sq'), 13: (12, 'mul'),
                  7: (6, 'mul'), 14: (7, 'sq'), 15: (14, 'mul')}
        PSLOT = {2: 'A', 4: 'B', 8: 'C', 16: 'D', 9: 'D', 5: 'C', 10: 'D',
                 11: 'B', 3: 'C', 6: 'A', 12: 'B', 13: 'D', 7: 'C', 14: 'B',
                 15: 'D'}
        PMULENG = {9: 'd', 5: 'p', 11: 'd', 3: 'p', 13: 'd', 7: 'p', 15: 'p'}

        def scan_emit(d, l, extra=()):
            S = LS[d]
            u, bcs, w = S["u"], S["bcs"], S["w"]
            pyt = py.tile([D_INNER, T], dt32, tag="py")
            S["pyt"] = pyt
            da_t, dbx_t, hs_t = {1: w}, {}, {}
            extra = list(extra)

            import os as _os2
            _dr = int(_os2.environ.get("BK_DRAIN", "1"))

            def drain():
                for _ in range(_dr):
                    if extra:
                        extra.pop(0)()

            def emit_da(idx):
                p = PORDER[idx]
                if p == 1:
                    return
                parent, op = PDERIV[p]
                da = s1.tile([D_INNER, T], dt16, tag=f"da{PSLOT[p]}",
                             name=f"da_p{p}")
                if op == 'sq':
                    nc.scalar.activation(da, da_t[parent], Act.Square)
                else:
                    eng = nc.vector if PMULENG[p] == 'd' else nc.gpsimd
                    eng.tensor_mul(da, da_t[parent], w)
                da_t[p] = da

            def emit_bside(idx):
                s = PORDER[idx] - 1
                mode = MUL_MODE[s]
                selB = selmat[0:D_STATE, s * D_INNER:(s + 1) * D_INNER]
                dbx = scn2.tile([D_INNER, T], dt16, tag="dbx")
                for j in range(T // MM):
                    sj = slice(j * MM, (j + 1) * MM)
                    bb = bc("bb")
                    nc.tensor.matmul(bb, selB, bcs[0:D_STATE, sj],
                                     start=True, stop=True)
                    if mode == "psum":
                        nc.vector.tensor_mul(dbx[:, sj], u[:, sj], bb)
                    else:
                        bb16 = scn.tile([D_INNER, MM], dt16, tag="bb16",
                                        name="bb16")
                        nc.scalar.activation(bb16, bb, Act.Copy)
                        eng = nc.gpsimd if mode == "pool" else nc.vector
                        eng.tensor_mul(dbx[:, sj], u[:, sj], bb16)
                dbx_t[idx] = dbx

            def emit_scan(idx):
                hs = scn2.tile([D_INNER, T], dt16, tag="hs")
                nc.vector.tensor_tensor_scan(hs, da_t[PORDER[idx]],
                                             dbx_t.pop(idx), 0.0,
                                             op0=Alu.mult, op1=Alu.add)
                hs_t[idx] = hs

            def emit_cside(idx):
                s = PORDER[idx] - 1
                mode = MUL_MODE[s]
                selC = selmat[32:32 + D_STATE, s * D_INNER:(s + 1) * D_INNER]
                hs = hs_t.pop(idx)
                for jp in range(T // MM // 2):
                    cbs_ = []
                    for j in (2 * jp, 2 * jp + 1):
                        sj = slice(j * MM, (j + 1) * MM)
                        cb_ = bc("cb")
                        nc.tensor.matmul(cb_, selC, bcs[32:32 + D_STATE, sj],
                                         start=True, stop=True)
                        cbs_.append((sj, cb_))
                    for sj, cb_ in cbs_:
                        if mode == "psum":
                            nc.vector.tensor_mul(hs[:, sj], hs[:, sj], cb_)
                        else:
                            cb16 = scn.tile([D_INNER, MM], dt16, tag="cb16",
                                            name="cb16")
                            nc.scalar.activation(cb16, cb_, Act.Copy)
                            eng = nc.gpsimd if mode == "pool" else nc.vector
                            eng.tensor_mul(hs[:, sj], hs[:, sj], cb16)
                    for sj, cb_ in cbs_:
                        nc.tensor.matmul(pyt[:, sj], ident, hs[:, sj],
                                         start=(idx == 0),
                                         stop=(idx == D_STATE - 1))

            emit_da(0)
            emit_bside(0)
            emit_da(1)
            emit_scan(0)
            for s in range(D_STATE):
                drain()
                if s + 1 < D_STATE:
                    emit_bside(s + 1)
                    if s + 2 < D_STATE:
                        emit_da(s + 2)
                    emit_scan(s + 1)
                emit_cside(s)
            while extra:
                extra.pop(0)()

        # post(d,l): y gate + out_proj + residual, as closures
        def post_stages(d, l, res_new):
            S = dict(LS[d])
            base = d * D_MODEL

            def gate():
                yo = s1.tile([D_INNER, T], dt16, tag=f"yo{d}", name=f"yo{d}")
                nc.vector.scalar_tensor_tensor(yo, S["xact"], P[("Dp", d, l)],
                                               S["pyt"], op0=Alu.mult,
                                               op1=Alu.add)
                nc.vector.tensor_mul(yo, yo, S["zsilu"])
                S["yo"] = yo

            def outp(j):
                sj = slice(j * MM, (j + 1) * MM)
                po = bc("po")
                nc.tensor.matmul(po[0:D_MODEL], P[("out_wT", d, l)],
                                 S["yo"][:, sj], start=True, stop=True)
                nc.vector.tensor_add(res_new[base:base + D_MODEL, sj],
                                     po[0:D_MODEL], S["src"][:, sj])

            return [gate] + [lambda j=j: outp(j) for j in range(T // MM)]

        # head(d): final LN -> softmax pool -> half of the final linear
        def head_stages(d, resb):
            base = d * D_MODEL
            hlnf = s1.tile([D_MODEL, T], dt16, tag=f"hln{d}", name=f"hlnf{d}")
            stages = ln_stages(resb[base:base + D_MODEL, :], nfw_sb, nfb_sb,
                               hlnf, f"p{d}")
            logits = hrow.tile([1, T], dt16, tag="logits", name=f"logits{d}")

            def lg(j):
                sj = slice(j * MM, (j + 1) * MM)
                pl = bc("pl")
                nc.tensor.matmul(pl[0:1], pw_sb[d], hlnf[:, sj],
                                 start=True, stop=True)
                nc.scalar.activation(logits[:, sj], pl[0:1], Act.Copy)

            for j in range(T // MM):
                stages.append(lambda j=j: lg(j))

            smalls = hrow.tile([1, 4], dt32, tag=f"smalls{d}")

            def softmax():
                nc.vector.reduce_max(smalls[:, 0:1], logits,
                                     axis=mybir.AxisListType.X)
                nc.vector.tensor_scalar_mul(smalls[:, 1:2], smalls[:, 0:1], -1.0)
                nc.scalar.activation(logits, logits, Act.Exp, bias=smalls[:, 1:2])
                nc.vector.reduce_sum(smalls[:, 2:3], logits,
                                     axis=mybir.AxisListType.X)
                nc.vector.reciprocal(smalls[:, 3:4], smalls[:, 2:3])
                nc.vector.tensor_scalar(logits, logits, smalls[:, 3:4], None,
                                        op0=Alu.mult)

            stages.append(softmax)
            acc = {}

            def pool(j):
                sj = slice(j * MM, (j + 1) * MM)
                ab = bc("ab")
                nc.tensor.matmul(ab[0:D_MODEL], ones_row, logits[:, sj],
                                 start=True, stop=True)
                scr2 = s1.tile([D_MODEL, MM], dt32, tag="poolscr")
                nc.vector.tensor_mul(scr2, hlnf[:, sj], ab[0:D_MODEL])
                pld = sb.tile([D_MODEL, 1], dt32, tag=f"pooled{d}")
                nc.vector.reduce_sum(pld, scr2, axis=mybir.AxisListType.X)
                if "p" in acc:
                    nc.vector.tensor_add(pld, pld, acc["p"])
                acc["p"] = pld

            for j in range(T // MM):
                stages.append(lambda j=j: pool(j))

            def fin():
                with nc.allow_low_precision("pooled bf16 feeds bf16 matmul"):
                    p16 = hrow.tile([D_MODEL, 1], dt16, tag=f"p16_{d}")
                    nc.vector.tensor_copy(p16, acc["p"])
                acc["p16"] = p16

            stages.append(fin)
            return stages, acc

        import os
        n_layers = int(os.environ.get("BK_LAYERS", N_LAYER))
        do_head = os.environ.get("BK_HEAD", "1") == "1"
        for rep in range(reps):
            if rep == 0:
                resb = resb0
            else:
                resb = sb.tile([2 * D_MODEL, T], dt32, tag="res")
                for d in range(2):
                    nc.sync.dma_start(
                        out=resb[d * D_MODEL:(d + 1) * D_MODEL, :], in_=xin[d])
            # software-pipelined two-stream schedule: d0's scan carries
            # d1's pre stages and vice versa across the layer boundary.
            for f in pre_stages(0, 0, resb):
                f()
            carry = []            # closures owed to the next scan emission
            res_news = {}
            acc0 = acc1 = None
            for l in range(n_layers):
                res_new = sb.tile([2 * D_MODEL, T], dt32, tag="res")
                res_news[l] = res_new
                scan_emit(0, l, extra=carry + pre_stages(1, l, resb))
                p0 = post_stages(0, l, res_new)
                if l + 1 < n_layers:
                    scan_emit(1, l, extra=p0 + pre_stages(0, l + 1, res_new))
                    carry = post_stages(1, l, res_new)
                else:
                    if do_head:
                        st0, acc0 = head_stages(0, res_new)
                    else:
                        st0 = []
                    scan_emit(1, l, extra=p0 + st0)
                    for f in post_stages(1, l, res_new):
                        f()
                resb = res_new

            if do_head:
                st1, acc1 = head_stages(1, resb)
                for f in st1:
                    f()
                pout = bc("pout")
                nc.tensor.matmul(pout[0:D_MODEL, 0:1], llw_sb[0], acc0["p16"],
                                 start=True, stop=False)
                nc.tensor.matmul(pout[0:D_MODEL, 0:1], llw_sb[1], acc1["p16"],
                                 start=False, stop=True)
                out_sb = hrow.tile([D_MODEL, 1], dt32, tag="outsb")
                nc.scalar.activation(out_sb, pout[0:D_MODEL, 0:1], Act.Identity,
                                     bias=llb_sb)
                nc.sync.dma_start(out=out_d, in_=out_sb)
            else:
                out_sb = hrow.tile([D_MODEL, 1], dt32, tag="outsb")
                nc.vector.tensor_copy(out_sb, resb[0:D_MODEL, 0:1])
                nc.sync.dma_start(out=out_d, in_=out_sb)

    if legalize:
        _legalize_sync_waits(nc, mybir)
    return nc


def _selmat():
    sel = np.zeros((48, D_STATE * D_INNER), np.float32)
    for s in range(D_STATE):
        sel[s, s * D_INNER:(s + 1) * D_INNER] = 1.0
        sel[32 + s, s * D_INNER:(s + 1) * D_INNER] = 1.0
    return sel


def prep_inputs(inputs):
    """Host-side prep: transposed bf16 weights as lhsT layouts, flipped input."""
    import ml_dtypes
    bf16 = ml_dtypes.bfloat16
    f = np.float32
    c = np.ascontiguousarray
    x = np.asarray(inputs["x"], f)               # [8, 64, 32, 64]
    xf = x.reshape(B, D_MODEL, T)                # feature-major [64, T]
    xb = xf[:, :, ::-1]
    xproj_wT = np.asarray(inputs["xproj_w"], f).transpose(0, 1, 3, 2)  # [2,4,128,36]
    xproj_pad = np.zeros((2, N_LAYER, D_INNER, 68), f)
    xproj_pad[..., 0:D_STATE] = xproj_wT[..., DT_RANK:DT_RANK + D_STATE]       # B
    xproj_pad[..., 32:32 + D_STATE] = xproj_wT[..., DT_RANK + D_STATE:]        # C
    xproj_pad[..., 64:68] = xproj_wT[..., 0:DT_RANK]                           # dt_raw
    ll_wT = np.asarray(inputs["ll_w"], f).T                            # [128, 64]
    common = {
        "in_wT": c(np.asarray(inputs["in_w"], f).transpose(0, 1, 3, 2)).astype(bf16),
        "conv_w": c(np.asarray(inputs["conv_w"], f)),
        "conv_b": c(np.asarray(inputs["conv_b"], f)[..., None]),
        "xproj_wTp": xproj_pad.astype(bf16),
        "dt_wT": c(np.asarray(inputs["dt_w"], f).transpose(0, 1, 3, 2)).astype(bf16),
        "dt_b": c(-np.asarray(inputs["dt_b"], f)[..., None]),
        "Dp": c(np.asarray(inputs["D"], f)[..., None]),
        "out_wT": c(np.asarray(inputs["out_w"], f).transpose(0, 1, 3, 2)).astype(bf16),
        "nw": c(np.asarray(inputs["nw"], f)[..., None]),
        "nb": c(np.asarray(inputs["nb"], f)[..., None]),
        "nf_w": c(np.asarray(inputs["nf_w"], f)[:, None]),
        "nf_b": c(np.asarray(inputs["nf_b"], f)[:, None]),
        "pool_wT": c(np.stack([np.asarray(inputs["fp_w"], f).T,
                               np.asarray(inputs["bp_w"], f).T])).astype(bf16),
        "ll_wT2": c(np.stack([ll_wT[0:D_MODEL], ll_wT[D_MODEL:]])).astype(bf16),
        "ll_b": c(np.asarray(inputs["ll_b"], f)[:, None]),
        "ident": np.eye(D_INNER, dtype=f).astype(bf16),
        "selmat": _selmat().astype(bf16),
    }
    in_maps = []
    for b in range(B):
        m = dict(common)
        m["xin"] = c(np.stack([xf[b], xb[b]]))
        in_maps.append(m)
    return in_maps


def kernel(**inputs):
    from concourse.bass_utils import run_bass_kernel_spmd
    in_maps = prep_inputs(inputs)
    nc = build_nc()
    res = run_bass_kernel_spmd(nc, in_maps, core_ids=list(range(NCORES)))
    out = np.stack([res.results[b]["out"][:, 0] for b in range(B)])
    return out.astype(np.float32)


# revision 39
# speedup vs baseline: 1.0606x; 1.0606x over previous
"""Bidirectional Mamba (MixerModel) Trainium2 kernel.

Sharding: data-parallel over batch. 8 batch elements -> 8 NeuronCores.
Each core runs the full 2-direction x 4-layer model for its batch element
(no collectives; the backward direction consumes a host-flipped copy of the
input, and the softmax attention pool is order-invariant so the backward
output never needs unflipping). Host stacks the per-core [64] outputs.

On-chip layout is feature-major: activations live as [feature, T] tiles so
the selective-scan recurrence h_t = dA_t * h_{t-1} + dBx_t maps onto the
DVE tensor_tensor_scan instruction (d_inner=128 on partitions, one scan
per state s=0..15). T=2048 is one chunk (no carries, no conv tail).

Decay factors via the model's exact A_s = -(s+1): da_s = exp(A_s*dt)
= w^(s+1) with w = sigmoid(-(dt_raw+dt_b)), so the 16 per-state decays
are built by repeated Act-engine squaring along a DFS of the power tree
(4 retention slots) plus 7 odd-power muls. Square/Copy live in EVERY
activation-table set, so the scan never forces a table reload
(ACT_TABLE_LOAD is 1.3us); the only per-layer excursion is one Ln for
u = dt*x = (-ln w)*x.

Schedule: the two direction streams are software-pipelined against each
other. Engine queues are in-order, so a lone serial stage chain starves
the machine; the emission order is

    ... scan(d0,l) [zipped with pre(d1,l)] ; post(d0,l) ;
        scan(d1,l) [zipped with post(d0,l), pre(d0,l+1)] ; ...

i.e. while one direction's 16-state scan occupies DVE/PE/Pool/Act, the
other direction's LN/in_proj/conv/x_proj closures are drained one per
state into the same queues. Within a scan, states are pipelined one
ahead (B-broadcast and scan of the next state issue before the C-side
of the current) so the PE never stalls behind the DVE.

Engine assignment (cost-model rates: DVE f32 1.04 / bf16 0.52 ns/col,
Act 0.93, Pool 2.03, PE bf16 0.83 ns/col): projections and one-hot row
broadcasts are bf16 PE matmuls into PSUM; the u*B / h*C muls either
read PSUM at DVE f32 rate ("psum" states) or are first evacuated
PSUM->SBUF bf16 by the Act engine — the real HW forbids GPSIMD PSUM
reads — and then multiplied on Pool or DVE-bf16 (MUL_MODE per state).
y = sum_s C_s*h_s accumulates on the PE via identity-matmul into a
[128,2048] PSUM tile. PSUM = py (4 banks) + 4x [128,512] "bc" ring
(4 banks) = exactly 8 banks.
"""

import numpy as np

D_MODEL = 64
N_LAYER = 4
D_INNER = 128
D_STATE = 16
D_CONV = 4
DT_RANK = 4
EPS = 1e-5
T = 2048
B = 8
NCORES = 8
MM = 512               # max matmul free dim (one PSUM bank)

# per-state mul routing for the scan's u*B / h*C products. The real HW
# forbids GPSIMD reads from PSUM, so Pool-assigned states first evacuate
# the PE broadcast PSUM->SBUF bf16 on the Act engine ("evac"), which also
# lets DVE run those muls at its 2x bf16 rate.
#   "psum": DVE mul reads the broadcast from PSUM (f32 rate, no evac)
#   "pool": Act evac -> Pool mul (SBUF)
#   "dve16": Act evac -> DVE mul (bf16 rate)
import os as _os
_cfg = _os.environ.get("BK_MULMODE", "psdpsdpsdpsdpsdp")
# per-state char: p=psum(DVE f32), s=pool(evac+Pool), d=dve16(evac+DVE bf16)
MUL_MODE = {_s: {"p": "psum", "s": "pool", "d": "dve16"}[_cfg[_s]]
            for _s in range(D_STATE)}


def _legalize_sync_waits(nc, mybir, maxw=None):
    import os
    if maxw is None:
        maxw = int(os.environ.get("BK_MAXW", "1"))
    """This container's walrus only accepts one sync-wait command per
    instruction (newer bass emits several, e.g. on the kernel-tail drain).
    Split excess waits onto preceding same-engine NOPs — semantically
    identical: the engine blocks on each wait in turn before the original
    instruction issues."""
    for blk in nc.m.functions[0].blocks:
        newlist, changed = [], False
        for inst in blk.instructions:
            si = inst.sync_info
            waits = list(si.on_wait) if si and si.on_wait else []
            if len(waits) > maxw:
                k = 0
                while len(waits) > maxw:
                    chunk, waits = waits[:maxw], waits[maxw:]
                    newlist.append(mybir.InstNoOp(
                        name=f"{inst.name}-waitsplit{k}", engine=inst.engine,
                        sync_info=mybir.SyncInfo(on_wait=chunk, on_update=[])))
                    k += 1
                inst.sync_info = mybir.SyncInfo(
                    on_wait=waits, on_update=list(si.on_update or []))
                changed = True
            newlist.append(inst)
        if changed:
            blk.instructions = newlist


def build_nc(legalize=True, reps=1):
    import concourse.bass as bass
    import concourse.bacc as bacc
    import concourse.mybir as mybir
    import concourse.tile as tile
    from contextlib import ExitStack

    dt32 = mybir.dt.float32
    dt16 = mybir.dt.bfloat16
    Alu = mybir.AluOpType
    Act = mybir.ActivationFunctionType

    # NOTE: bacc.Bacc's finalize pipeline emits event-semaphore/register
    # constructs this container's walrus rejects ("Reg has not been
    # allocated"), so we stay on plain Bass + _legalize_sync_waits.
    nc = bass.Bass("TRN2", target_bir_lowering=False, debug=False,
                   num_devices=NCORES)

    def din(name, shape, dt=dt32):
        return nc.dram_tensor(name, list(shape), dt, kind="ExternalInput").ap()

    xin = din("xin", (2, D_MODEL, T))           # fwd + flipped input
    in_wT = din("in_wT", (2, N_LAYER, D_MODEL, 2 * D_INNER), dt16)
    conv_w = din("conv_w", (2, N_LAYER, D_INNER, D_CONV))
    conv_b = din("conv_b", (2, N_LAYER, D_INNER, 1))
    xproj_wTp = din("xproj_wTp", (2, N_LAYER, D_INNER, 68), dt16)
    dt_wT = din("dt_wT", (2, N_LAYER, DT_RANK, D_INNER), dt16)
    dt_b = din("dt_b", (2, N_LAYER, D_INNER, 1))
    D_in = din("Dp", (2, N_LAYER, D_INNER, 1))
    out_wT = din("out_wT", (2, N_LAYER, D_INNER, D_MODEL), dt16)
    nw_in = din("nw", (2, N_LAYER, D_MODEL, 1))
    nb_in = din("nb", (2, N_LAYER, D_MODEL, 1))
    nf_w = din("nf_w", (D_MODEL, 1))
    nf_b = din("nf_b", (D_MODEL, 1))
    pool_wT = din("pool_wT", (2, D_MODEL, 1), dt16)
    ll_wT2 = din("ll_wT2", (2, D_MODEL, D_MODEL), dt16)
    ll_b = din("ll_b", (D_MODEL, 1))
    ident_in = din("ident", (D_INNER, D_INNER), dt16)
    selmat_in = din("selmat", (48, D_STATE * D_INNER), dt16)

    out_d = nc.dram_tensor("out", [D_MODEL, 1], dt32, kind="ExternalOutput").ap()

    with tile.TileContext(nc) as tc, ExitStack() as ctx:
        const = ctx.enter_context(tc.tile_pool(name="const", bufs=1))
        sb = ctx.enter_context(tc.tile_pool(name="sb", bufs=2))
        s1 = ctx.enter_context(tc.tile_pool(name="s1", bufs=1))
        scn = ctx.enter_context(tc.tile_pool(name="scn", bufs=2))
        scn2 = ctx.enter_context(tc.tile_pool(name="scn2", bufs=2))
        rows = ctx.enter_context(tc.tile_pool(name="rows", bufs=1))
        hrow = ctx.enter_context(tc.tile_pool(name="hrow", bufs=1))
        # PSUM: 4-deep ring of [128,512] tiles (1 bank each) for every
        # projection output / broadcast, + the [128,2048] y-accumulator.
        ps = ctx.enter_context(tc.tile_pool(name="ps", bufs=4, space="PSUM"))
        py = ctx.enter_context(tc.tile_pool(name="py", bufs=1, space="PSUM"))

        def bc(name):
            return ps.tile([D_INNER, MM], dt32, tag="bc", name=name)

        ones_row = const.tile([1, D_MODEL], dt16, tag="ones_row")
        nc.vector.memset(ones_row, 1.0)
        lnsel = const.tile([D_MODEL, 1], dt16, tag="lnsel")
        nc.vector.memset(lnsel, 1.0 / D_MODEL)
        eps_c = const.tile([1, 1], dt32, tag="epsc")
        nc.vector.memset(eps_c, EPS)
        negone = const.tile([D_INNER, 1], dt32, tag="negone")
        nc.vector.memset(negone, -1.0)

        _dmaq = [0]

        def cload(tag, ap_src, shape, dt=dt32):
            t = const.tile(list(shape), dt, tag=tag)
            # alternate the two HWDGE trigger queues (SP / Activation)
            q = nc.sync if _dmaq[0] % 2 == 0 else nc.scalar
            _dmaq[0] += 1
            q.dma_start(out=t, in_=ap_src)
            return t

        # rep-0 input load first: the first LN depends on it, and the bulk
        # weight DMAs would otherwise delay kernel start by ~50us.
        resb0 = sb.tile([2 * D_MODEL, T], dt32, tag="res", name="resb0")
        for d in range(2):
            nc.sync.dma_start(out=resb0[d * D_MODEL:(d + 1) * D_MODEL, :],
                              in_=xin[d])

        P = {}
        SI = {}
        for l in range(N_LAYER):
            if l == 1:
                SI["ident"] = const.tile([D_INNER, D_INNER], dt16, tag="ident", name="ident")
                nc.sync.dma_start(out=SI["ident"], in_=ident_in)
                SI["selmat"] = const.tile([48, D_STATE * D_INNER], dt16,
                                          tag="selmat", name="selmat")
                nc.scalar.dma_start(out=SI["selmat"], in_=selmat_in)
            for d in range(2):
                k = (d, l)
                P[("in_wT",) + k] = cload(f"in_wT{d}{l}", in_wT[d, l], (D_MODEL, 2 * D_INNER), dt16)
                P[("conv_w",) + k] = cload(f"conv_w{d}{l}", conv_w[d, l], (D_INNER, D_CONV))
                P[("conv_b",) + k] = cload(f"conv_b{d}{l}", conv_b[d, l], (D_INNER, 1))
                P[("xproj_wTp",) + k] = cload(f"xproj{d}{l}", xproj_wTp[d, l], (D_INNER, 68), dt16)
                P[("dt_wT",) + k] = cload(f"dtw{d}{l}", dt_wT[d, l], (DT_RANK, D_INNER), dt16)
                P[("dt_b",) + k] = cload(f"dt_b{d}{l}", dt_b[d, l], (D_INNER, 1))
                P[("Dp",) + k] = cload(f"Dp{d}{l}", D_in[d, l], (D_INNER, 1))
                P[("out_wT",) + k] = cload(f"out_wT{d}{l}", out_wT[d, l], (D_INNER, D_MODEL), dt16)
                P[("nw",) + k] = cload(f"nw{d}{l}", nw_in[d, l], (D_MODEL, 1))
                P[("nb",) + k] = cload(f"nb{d}{l}", nb_in[d, l], (D_MODEL, 1))
        ident = SI["ident"]
        selmat = SI["selmat"]
        nfw_sb = cload("nfw", nf_w, (D_MODEL, 1))
        nfb_sb = cload("nfb", nf_b, (D_MODEL, 1))
        pw_sb = [cload(f"pw{d}", pool_wT[d], (D_MODEL, 1), dt16) for d in range(2)]
        llw_sb = [cload(f"llw{d}", ll_wT2[d], (D_MODEL, D_MODEL), dt16) for d in range(2)]
        llb_sb = cload("llb", ll_b, (D_MODEL, 1))

        # ---- layernorm over features -> list of emission closures ------
        # src [64,T] f32 at base 0/64; writes hln bf16 (base 0).
        def ln_stages(src, nw_c, nb_c, hln_c, tp):
            def stat():
                src16 = s1.tile([D_MODEL, T], dt16, tag=f"lnsrc{tp}", name="src16")
                nc.scalar.activation(src16, src, Act.Copy)
                sq16 = s1.tile([D_MODEL, T], dt16, tag=f"lnsq{tp}", name="sq16")
                nc.scalar.activation(sq16, src, Act.Square)
                mean_sb = rows.tile([1, T], dt16, tag=f"mean{tp}")
                rstd_sb = rows.tile([1, T], dt16, tag=f"rstd{tp}")
                scr = rows.tile([1, T], dt32, tag=f"scr{tp}")
                for j in range(T // MM):
                    sj = slice(j * MM, (j + 1) * MM)
                    pm = bc("pmln")
                    nc.tensor.matmul(pm[0:1], lnsel, src16[:, sj],
                                     start=True, stop=True)
                    nc.tensor.matmul(pm[32:33], lnsel, sq16[:, sj],
                                     start=True, stop=True)
                    nc.scalar.activation(mean_sb[:, sj], pm[0:1], Act.Copy)
                    nc.scalar.activation(scr[:, sj], pm[32:33], Act.Copy)
                # rstd doubles as mean^2 scratch before its final value
                with nc.allow_low_precision("rstd bf16 feeds bf16 matmul"):
                    nc.scalar.activation(rstd_sb, mean_sb, Act.Square)
                    nc.vector.tensor_sub(scr, scr, rstd_sb)
                    nc.scalar.activation(scr, scr, Act.Sqrt, bias=eps_c)
                    nc.vector.reciprocal(rstd_sb, scr)
                return mean_sb, rstd_sb

            def norm(mr, j):
                mean_sb, rstd_sb = mr
                sj = slice(j * MM, (j + 1) * MM)
                mb = bc("mbln")
                nc.tensor.matmul(mb[0:D_MODEL], ones_row, mean_sb[:, sj],
                                 start=True, stop=True)
                rb = bc("rbln")
                nc.tensor.matmul(rb[0:D_MODEL], ones_row, rstd_sb[:, sj],
                                 start=True, stop=True)
                tmp = s1.tile([D_MODEL, MM], dt32, tag=f"lntmp{tp}")
                nc.vector.tensor_sub(tmp, src[:, sj], mb[0:D_MODEL])
                nc.vector.scalar_tensor_tensor(tmp, tmp, nw_c, rb[0:D_MODEL],
                                               op0=Alu.mult, op1=Alu.mult)
                nc.scalar.activation(hln_c[:, sj], tmp, Act.Identity, bias=nb_c)

            st = {}
            stages = [lambda: st.__setitem__('mr', stat())]
            for j in range(T // MM):
                stages.append(lambda j=j: norm(st['mr'], j))
            return stages

        # ---- per-direction layer state ---------------------------------
        LS = [{}, {}]

        # pre(d,l): LN -> in_proj -> conv -> x_proj -> dt -> u, as closures
        def pre_stages(d, l, res_old):
            S = LS[d]
            base = d * D_MODEL
            src = res_old[base:base + D_MODEL, :]
            stages = []

            hln = s1.tile([D_MODEL, T], dt16, tag=f"hln{d}", name=f"hln{d}")
            S["src"] = src
            stages += ln_stages(src, P[("nw", d, l)], P[("nb", d, l)], hln, f"p{d}")

            xpad = s1.tile([D_INNER, D_CONV - 1 + T], dt16, tag=f"xpad{d}",
                           name=f"xpad{d}")
            zsilu = s1.tile([D_INNER, T], dt16, tag=f"zsilu{d}", name=f"zsilu{d}")
            S["zsilu"] = zsilu
            wx = P[("in_wT", d, l)]

            def inproj(j):
                sj = slice(j * MM, (j + 1) * MM)
                px = bc("px")
                nc.tensor.matmul(px, wx[:, 0:D_INNER], hln[:, sj],
                                 start=True, stop=True)
                nc.scalar.activation(
                    xpad[:, D_CONV - 1 + j * MM:D_CONV - 1 + (j + 1) * MM],
                    px, Act.Copy)
                pz = bc("pz")
                nc.tensor.matmul(pz, wx[:, D_INNER:], hln[:, sj],
                                 start=True, stop=True)
                zsig = sb.tile([D_INNER, MM], dt16, tag="zsig")
                nc.scalar.activation(zsig, pz, Act.Sigmoid)
                nc.vector.tensor_mul(zsilu[:, sj], zsig, pz)

            for j in range(T // MM):
                stages.append(lambda j=j: inproj(j))

            cw = P[("conv_w", d, l)]
            cacc = s1.tile([D_INNER, T], dt16, tag=f"yo{d}", name=f"cacc{d}")
            xact = s1.tile([D_INNER, T], dt16, tag=f"xact{d}", name=f"xact{d}")
            S["xact"] = xact

            def conv_a():
                nc.vector.memset(xpad[:, 0:D_CONV - 1], 0.0)
                nc.gpsimd.tensor_scalar(cacc, xpad[:, 0:T], cw[:, 0:1],
                                        P[("conv_b", d, l)],
                                        op0=Alu.mult, op1=Alu.add)

            def conv_tap(jj):
                nc.vector.scalar_tensor_tensor(cacc, xpad[:, jj:jj + T],
                                               cw[:, jj:jj + 1], cacc,
                                               op0=Alu.mult, op1=Alu.add)

            def conv_c():
                xsig = s1.tile([D_INNER, T], dt16, tag=f"xsig{d}", name=f"xsig{d}")
                nc.scalar.activation(xsig, cacc, Act.Sigmoid)
                nc.gpsimd.tensor_mul(xact, cacc, xsig)

            stages.append(conv_a)
            for jj in range(1, D_CONV):
                stages.append(lambda jj=jj: conv_tap(jj))
            stages.append(conv_c)

            bcs = s1.tile([48, T], dt16, tag=f"bcs{d}", name=f"bcs{d}")
            dtr = s1.tile([DT_RANK, T], dt16, tag=f"dtr{d}", name=f"dtr{d}")
            S["bcs"] = bcs

            def xproj(j):
                sj = slice(j * MM, (j + 1) * MM)
                pd_ = bc("pd")
                nc.tensor.matmul(pd_[0:68], P[("xproj_wTp", d, l)], xact[:, sj],
                                 start=True, stop=True)
                nc.scalar.activation(bcs[:, sj], pd_[0:48], Act.Copy)
                nc.scalar.activation(dtr[:, sj], pd_[64:68], Act.Copy)

            for j in range(T // MM):
                stages.append(lambda j=j: xproj(j))

            # w = sigmoid(-(dt_raw + dt_b)) = exp(-softplus(dt_raw + dt_b))
            # = exp(-dt): the per-state decay is da_s = w^(s+1) (A_s = -(s+1)
            # exactly in this model), built by repeated squaring -- Square is
            # in EVERY act-table set, so the scan causes no table reloads.
            # dt_b arrives negated from the host.
            w = s1.tile([D_INNER, T], dt16, tag=f"w{d}", name=f"w{d}")
            S["w"] = w

            def dtproj(j):
                sj = slice(j * MM, (j + 1) * MM)
                pt = bc("pt")
                nc.tensor.matmul(pt, P[("dt_wT", d, l)], dtr[:, sj],
                                 start=True, stop=True)
                nc.scalar.activation(w[:, sj], pt, Act.Sigmoid, scale=-1.0,
                                     bias=P[("dt_b", d, l)])

            for j in range(T // MM):
                stages.append(lambda j=j: dtproj(j))

            def umul():
                # u = dt * x = (-ln w) * x  (the layer's one Ln excursion)
                u = s1.tile([D_INNER, T], dt16, tag=f"u{d}", name=f"u{d}")
                nc.scalar.activation(u, w, Act.Ln)
                nc.vector.scalar_tensor_tensor(u, u, negone, xact,
                                               op0=Alu.mult, op1=Alu.mult)
                S["u"] = u

            stages.append(umul)
            return stages

        # scan(d,l): 16-state selective scan, one state pipelined ahead.
        # `extra` closures (the other stream's stages) are drained one per
        # state so both streams share the engine queues.
        # da_s = w^(s+1): power tree visited in DFS order so each tile is
        # derived from a live parent by Act.Square (universal table set) or
        # a DVE/Pool mul with w. 4 retention slots cover peak liveness.
        PORDER = [1, 2, 4, 8, 16, 9, 5, 10, 11, 3, 6, 12, 13, 7, 14, 15]
        PDERIV = {2: (1, 'sq'), 4: (2, 'sq'), 8: (4, 'sq'), 16: (8, 'sq'),
                  9: (8, 'mul'), 5: (4, 'mul'), 10: (5, 'sq'), 11: (10, 'mul'),
                  3: (2, 'mul'), 6: (3, 'sq'), 12: (6, 'sq'), 13: (12, 'mul'),
                  7: (6, 'mul'), 14: (7, 'sq'), 15: (14, 'mul')}
        PSLOT = {2: 'A', 4: 'B', 8: 'C', 16: 'D', 9: 'D', 5: 'C', 10: 'D',
                 11: 'B', 3: 'C', 6: 'A', 12: 'B', 13: 'D', 7: 'C', 14: 'B',
                 15: 'D'}
        PMULENG = {9: 'd', 5: 'p', 11: 'd', 3: 'p', 13: 'd', 7: 'p', 15: 'p'}

        def scan_emit(d, l, extra=()):
            S = LS[d]
            u, bcs, w = S["u"], S["bcs"], S["w"]
            pyt = py.tile([D_INNER, T], dt32, tag="py")
            S["pyt"] = pyt
            da_t, dbx_t, hs_t = {1: w}, {}, {}
            extra = list(extra)

            import os as _os2
            _dr = int(_os2.environ.get("BK_DRAIN", "1"))

            def drain():
                for _ in range(_dr):
                    if extra:
                        extra.pop(0)()

            def emit_da(idx):
                p = PORDER[idx]
                if p == 1:
                    return
                parent, op = PDERIV[p]
                da = s1.tile([D_INNER, T], dt16, tag=f"da{PSLOT[p]}",
                             name=f"da_p{p}")
                if op == 'sq':
                    nc.scalar.activation(da, da_t[parent], Act.Square)
                else:
                    eng = nc.vector if PMULENG[p] == 'd' else nc.gpsimd
                    eng.tensor_mul(da, da_t[parent], w)
                da_t[p] = da

            def emit_bside(idx):
                s = PORDER[idx] - 1
                mode = MUL_MODE[s]
                selB = selmat[0:D_STATE, s * D_INNER:(s + 1) * D_INNER]
                dbx = scn2.tile([D_INNER, T], dt16, tag="dbx")
                for j in range(T // MM):
                    sj = slice(j * MM, (j + 1) * MM)
                    bb = bc("bb")
                    nc.tensor.matmul(bb, selB, bcs[0:D_STATE, sj],
                                     start=True, stop=True)
                    if mode == "psum":
                        nc.vector.tensor_mul(dbx[:, sj], u[:, sj], bb)
                    else:
                        bb16 = scn.tile([D_INNER, MM], dt16, tag="bb16",
                                        name="bb16")
                        nc.scalar.activation(bb16, bb, Act.Copy)
                        eng = nc.gpsimd if mode == "pool" else nc.vector
                        eng.tensor_mul(dbx[:, sj], u[:, sj], bb16)
                dbx_t[idx] = dbx

            def emit_scan(idx):
                hs = scn2.tile([D_INNER, T], dt16, tag="hs")
                nc.vector.tensor_tensor_scan(hs, da_t[PORDER[idx]],
                                             dbx_t.pop(idx), 0.0,
                                             op0=Alu.mult, op1=Alu.add)
                hs_t[idx] = hs

            def emit_cside(idx):
                s = PORDER[idx] - 1
                mode = MUL_MODE[s]
                selC = selmat[32:32 + D_STATE, s * D_INNER:(s + 1) * D_INNER]
                hs = hs_t.pop(idx)
                for jp in range(T // MM // 2):
                    cbs_ = []
                    for j in (2 * jp, 2 * jp + 1):
                        sj = slice(j * MM, (j + 1) * MM)
                        cb_ = bc("cb")
                        nc.tensor.matmul(cb_, selC, bcs[32:32 + D_STATE, sj],
                                         start=True, stop=True)
                        cbs_.append((sj, cb_))
                    for sj, cb_ in cbs_:
                        if mode == "psum":
                            nc.vector.tensor_mul(hs[:, sj], hs[:, sj], cb_)
                        else:
                            cb16 = scn.tile([D_INNER, MM], dt16, tag="cb16",
                                            name="cb16")
                            nc.scalar.activation(cb16, cb_, Act.Copy)
                            eng = nc.gpsimd if mode == "pool" else nc.vector
                            eng.tensor_mul(hs[:, sj], hs[:, sj], cb16)
                    for sj, cb_ in cbs_:
                        nc.tensor.matmul(pyt[:, sj], ident, hs[:, sj],
                                         start=(idx == 0),
                                         stop=(idx == D_STATE - 1))

            emit_da(0)
            emit_bside(0)
            emit_da(1)
            emit_scan(0)
            for s in range(D_STATE):
                drain()
                if s + 1 < D_STATE:
                    emit_bside(s + 1)
                    if s + 2 < D_STATE:
                        emit_da(s + 2)
                    emit_scan(s + 1)
                emit_cside(s)
            while extra:
                extra.pop(0)()

        # post(d,l): y gate + out_proj + residual, as closures
        def post_stages(d, l, res_new):
            S = dict(LS[d])
            base = d * D_MODEL

            def gate():
                yo = s1.tile([D_INNER, T], dt16, tag=f"yo{d}", name=f"yo{d}")
                nc.vector.scalar_tensor_tensor(yo, S["xact"], P[("Dp", d, l)],
                                               S["pyt"], op0=Alu.mult,
                                               op1=Alu.add)
                nc.vector.tensor_mul(yo, yo, S["zsilu"])
                S["yo"] = yo

            def outp(j):
                sj = slice(j * MM, (j + 1) * MM)
                po = bc("po")
                nc.tensor.matmul(po[0:D_MODEL], P[("out_wT", d, l)],
                                 S["yo"][:, sj], start=True, stop=True)
                nc.vector.tensor_add(res_new[base:base + D_MODEL, sj],
                                     po[0:D_MODEL], S["src"][:, sj])

            return [gate] + [lambda j=j: outp(j) for j in range(T // MM)]

        # head(d): final LN -> softmax pool -> half of the final linear
        def head_stages(d, resb):
            base = d * D_MODEL
            hlnf = s1.tile([D_MODEL, T], dt16, tag=f"hln{d}", name=f"hlnf{d}")
            stages = ln_stages(resb[base:base + D_MODEL, :], nfw_sb, nfb_sb,
                               hlnf, f"p{d}")
            logits = hrow.tile([1, T], dt16, tag="logits", name=f"logits{d}")

            def lg(j):
                sj = slice(j * MM, (j + 1) * MM)
                pl = bc("pl")
                nc.tensor.matmul(pl[0:1], pw_sb[d], hlnf[:, sj],
                                 start=True, stop=True)
                nc.scalar.activation(logits[:, sj], pl[0:1], Act.Copy)

            for j in range(T // MM):
                stages.append(lambda j=j: lg(j))

            smalls = hrow.tile([1, 4], dt32, tag=f"smalls{d}")

            def softmax():
                nc.vector.reduce_max(smalls[:, 0:1], logits,
                                     axis=mybir.AxisListType.X)
                nc.vector.tensor_scalar_mul(smalls[:, 1:2], smalls[:, 0:1], -1.0)
                nc.scalar.activation(logits, logits, Act.Exp, bias=smalls[:, 1:2])
                nc.vector.reduce_sum(smalls[:, 2:3], logits,
                                     axis=mybir.AxisListType.X)
                nc.vector.reciprocal(smalls[:, 3:4], smalls[:, 2:3])
                nc.vector.tensor_scalar(logits, logits, smalls[:, 3:4], None,
                                        op0=Alu.mult)

            stages.append(softmax)
            acc = {}

            def pool(j):
                sj = slice(j * MM, (j + 1) * MM)
                ab = bc("ab")
                nc.tensor.matmul(ab[0:D_MODEL], ones_row, logits[:, sj],
                                 start=True, stop=True)
                scr2 = s1.tile([D_MODEL, MM], dt32, tag="poolscr")
                nc.vector.tensor_mul(scr2, hlnf[:, sj], ab[0:D_MODEL])
                pld = sb.tile([D_MODEL, 1], dt32, tag=f"pooled{d}")
                nc.vector.reduce_sum(pld, scr2, axis=mybir.AxisListType.X)
                if "p" in acc:
                    nc.vector.tensor_add(pld, pld, acc["p"])
                acc["p"] = pld

            for j in range(T // MM):
                stages.append(lambda j=j: pool(j))

            def fin():
                with nc.allow_low_precision("pooled bf16 feeds bf16 matmul"):
                    p16 = hrow.tile([D_MODEL, 1], dt16, tag=f"p16_{d}")
                    nc.vector.tensor_copy(p16, acc["p"])
                acc["p16"] = p16

            stages.append(fin)
            return stages, acc

        import os
        n_layers = int(os.environ.get("BK_LAYERS", N_LAYER))
        do_head = os.environ.get("BK_HEAD", "1") == "1"
        for rep in range(reps):
            if rep == 0:
                resb = resb0
            else:
                resb = sb.tile([2 * D_MODEL, T], dt32, tag="res")
                for d in range(2):
                    nc.sync.dma_start(
                        out=resb[d * D_MODEL:(d + 1) * D_MODEL, :], in_=xin[d])
            # software-pipelined two-stream schedule: d0's scan carries
            # d1's pre stages and vice versa across the layer boundary.
            for f in pre_stages(0, 0, resb):
                f()
            carry = []            # closures owed to the next scan emission
            res_news = {}
            acc0 = acc1 = None
            for l in range(n_layers):
                res_new = sb.tile([2 * D_MODEL, T], dt32, tag="res")
                res_news[l] = res_new
                scan_emit(0, l, extra=carry + pre_stages(1, l, resb))
                p0 = post_stages(0, l, res_new)
                if l + 1 < n_layers:
                    scan_emit(1, l, extra=p0 + pre_stages(0, l + 1, res_new))
                    carry = post_stages(1, l, res_new)
                else:
                    if do_head:
                        st0, acc0 = head_stages(0, res_new)
                    else:
                        st0 = []
                    scan_emit(1, l, extra=p0 + st0)
                    for f in post_stages(1, l, res_new):
                        f()
                resb = res_new

            if do_head:
                st1, acc1 = head_stages(1, resb)
                for f in st1:
                    f()
                pout = bc("pout")
                nc.tensor.matmul(pout[0:D_MODEL, 0:1], llw_sb[0], acc0["p16"],
                                 start=True, stop=False)
                nc.tensor.matmul(pout[0:D_MODEL, 0:1], llw_sb[1], acc1["p16"],
                                 start=False, stop=True)
                out_sb = hrow.tile([D_MODEL, 1], dt32, tag="outsb")
                nc.scalar.activation(out_sb, pout[0:D_MODEL, 0:1], Act.Identity,
                                     bias=llb_sb)
                nc.sync.dma_start(out=out_d, in_=out_sb)
            else:
                out_sb = hrow.tile([D_MODEL, 1], dt32, tag="outsb")
                nc.vector.tensor_copy(out_sb, resb[0:D_MODEL, 0:1])
                nc.sync.dma_start(out=out_d, in_=out_sb)

    if legalize:
        _legalize_sync_waits(nc, mybir)
    return nc


def _selmat():
    sel = np.zeros((48, D_STATE * D_INNER), np.float32)
    for s in range(D_STATE):
        sel[s, s * D_INNER:(s + 1) * D_INNER] = 1.0
        sel[32 + s, s * D_INNER:(s + 1) * D_INNER] = 1.0
    return sel


def prep_inputs(inputs):
    """Host-side prep: transposed bf16 weights as lhsT layouts, flipped input."""
    import ml_dtypes
    bf16 = ml_dtypes.bfloat16
    f = np.float32
    c = np.ascontiguousarray
    x = np.asarray(inputs["x"], f)               # [8, 64, 32, 64]
    xf = x.reshape(B, D_MODEL, T)                # feature-major [64, T]
    xb = xf[:, :, ::-1]
    xproj_wT = np.asarray(inputs["xproj_w"], f).transpose(0, 1, 3, 2)  # [2,4,128,36]
    xproj_pad = np.zeros((2, N_LAYER, D_INNER, 68), f)
    xproj_pad[..., 0:D_STATE] = xproj_wT[..., DT_RANK:DT_RANK + D_STATE]       # B
    xproj_pad[..., 32:32 + D_STATE] = xproj_wT[..., DT_RANK + D_STATE:]        # C
    xproj_pad[..., 64:68] = xproj_wT[..., 0:DT_RANK]                           # dt_raw
    ll_wT = np.asarray(inputs["ll_w"], f).T                            # [128, 64]
    common = {
        "in_wT": c(np.asarray(inputs["in_w"], f).transpose(0, 1, 3, 2)).astype(bf16),
        "conv_w": c(np.asarray(inputs["conv_w"], f)),
        "conv_b": c(np.asarray(inputs["conv_b"], f)[..., None]),
        "xproj_wTp": xproj_pad.astype(bf16),
        "dt_wT": c(np.asarray(inputs["dt_w"], f).transpose(0, 1, 3, 2)).astype(bf16),
        "dt_b": c(-np.asarray(inputs["dt_b"], f)[..., None]),
        "Dp": c(np.asarray(inputs["D"], f)[..., None]),
        "out_wT": c(np.asarray(inputs["out_w"], f).transpose(0, 1, 3, 2)).astype(bf16),
        "nw": c(np.asarray(inputs["nw"], f)[..., None]),
        "nb": c(np.asarray(inputs["nb"], f)[..., None]),
        "nf_w": c(np.asarray(inputs["nf_w"], f)[:, None]),
        "nf_b": c(np.asarray(inputs["nf_b"], f)[:, None]),
        "pool_wT": c(np.stack([np.asarray(inputs["fp_w"], f).T,
                               np.asarray(inputs["bp_w"], f).T])).astype(bf16),
        "ll_wT2": c(np.stack([ll_wT[0:D_MODEL], ll_wT[D_MODEL:]])).astype(bf16),
        "ll_b": c(np.asarray(inputs["ll_b"], f)[:, None]),
        "ident": np.eye(D_INNER, dtype=f).astype(bf16),
        "selmat": _selmat().astype(bf16),
    }
    in_maps = []
    for b in range(B):
        m = dict(common)
        m["xin"] = c(np.stack([xf[b], xb[b]]))
        in_maps.append(m)
    return in_maps


def kernel(**inputs):
    from concourse.bass_utils import run_bass_kernel_spmd
    in_maps = prep_inputs(inputs)
    nc = build_nc()
    res = run_bass_kernel_spmd(nc, in_maps, core_ids=list(range(NCORES)))
    out = np.stack([res.results[b]["out"][:, 0] for b in range(B)])
    return out.astype(np.float32)
